# revision 8
# baseline (speedup 1.0000x reference)
"""Multi-head differential attention on 8 Trainium2 NeuronCores.

Sharding: core c -> batch c//4, head-group c%4 (4 of 16 heads).
Each core: QKV projection for its heads, attention, per-(batch,head)
GroupNorm, partial out-projection in transposed (yT) layout, then an
on-device AllReduce over the 4 cores that share a batch.

Host side folds: lambda (differential-attention scalar per head) and the
softmax scale into Wq/bq; GroupNorm affine (gn_w, gn_b) into Wo/bo.
Inputs are pre-transposed (x -> xT per batch) and cast to bf16 on host so
all device matmuls run at 1 cycle/row.
"""

import numpy as np
import ml_dtypes

B, S, D, H, DH = 2, 2048, 1024, 16, 64
HPC = 4            # heads per core
CW = HPC * DH      # columns per core (256)
EPS = 1e-5
LAMBDA_INIT = 0.8
N_CORES = 8

_cache = {}


def _build(with_collective=True):
    from contextlib import ExitStack
    import concourse.bass as bass
    from concourse import bacc
    import concourse.tile as tile
    import concourse.mybir as mybir

    f32 = mybir.dt.float32
    bf16 = mybir.dt.bfloat16
    AF = mybir.ActivationFunctionType
    ALU = mybir.AluOpType

    nc = bacc.Bacc("TRN2", target_bir_lowering=False, debug=False, num_devices=N_CORES)

    xt_d = nc.dram_tensor("xt", [D, S], bf16, kind="ExternalInput")
    wq_d = nc.dram_tensor("wq", [D, CW], bf16, kind="ExternalInput")
    wk_d = nc.dram_tensor("wk", [D, CW], bf16, kind="ExternalInput")
    wv_d = nc.dram_tensor("wv", [D, CW], bf16, kind="ExternalInput")
    wo_d = nc.dram_tensor("wo", [CW, D], bf16, kind="ExternalInput")
    bq_d = nc.dram_tensor("bq", [CW], bf16, kind="ExternalInput")
    bk_d = nc.dram_tensor("bk", [CW], bf16, kind="ExternalInput")
    bv_d = nc.dram_tensor("bv", [CW], f32, kind="ExternalInput")
    bo_d = nc.dram_tensor("bo", [D], bf16, kind="ExternalInput")
    y_d = nc.dram_tensor("y", [8, 128, S], f32, kind="ExternalOutput")

    cc_in = nc.dram_tensor("cc_in", [8, 128, S], f32)
    cc_out = nc.dram_tensor("cc_out", [8, 128, S], f32)
    rs_d = nc.dram_tensor("rs_scratch", [16, 512], f32)     # (h*4+qt, q)
    rc_d = nc.dram_tensor("rc_scratch", [HPC, S], f32)      # recip rowsums
    sc_d = nc.dram_tensor("sc_scratch", [8], f32)           # M[4], rstd[4]

    NQT = 4          # query tiles of 512
    QT = 512
    NKT = 16         # key tiles of 128
    NDC = 8          # d-chunks of 128

    with ExitStack() as ctx:
        tc = ctx.enter_context(tile.TileContext(nc))
        const = ctx.enter_context(tc.tile_pool(name="const", bufs=1))
        big = ctx.enter_context(tc.tile_pool(name="big", bufs=1))

        # ---- constant / persistent SBUF ----
        xt_sb = big.tile([128, NDC, S], bf16)
        nc.sync.dma_start(out=xt_sb, in_=xt_d[:, :].rearrange("(c p) s -> p c s", p=128))

        wq_sb = const.tile([128, NDC, CW], bf16, tag="wq")
        wk_sb = const.tile([128, NDC, CW], bf16, tag="wk")
        wv_sb = const.tile([128, NDC, CW], bf16, tag="wv")
        nc.sync.dma_start(out=wq_sb, in_=wq_d[:, :].rearrange("(c p) n -> p c n", p=128))
        nc.sync.dma_start(out=wk_sb, in_=wk_d[:, :].rearrange("(c p) n -> p c n", p=128))
        nc.sync.dma_start(out=wv_sb, in_=wv_d[:, :].rearrange("(c p) n -> p c n", p=128))
        # wo as pair-stacked [128 = (o p), t, D] so out-proj contracts K=128
        wo_sb = const.tile([128, 2, D], bf16, tag="wo")
        nc.sync.dma_start(out=wo_sb, in_=wo_d[:, :].rearrange("(t o p) n -> (o p) t n", t=2, o=2, p=64))

        bqr_sb = const.tile([1, CW], bf16, tag="bqr")
        bkr_sb = const.tile([1, CW], bf16, tag="bkr")
        bor_sb = const.tile([1, D], bf16, tag="bor")
        nc.sync.dma_start(out=bqr_sb, in_=bq_d[:].rearrange("(a n) -> a n", a=1))
        nc.sync.dma_start(out=bkr_sb, in_=bk_d[:].rearrange("(a n) -> a n", a=1))
        nc.sync.dma_start(out=bor_sb, in_=bo_d[:].rearrange("(a n) -> a n", a=1))
        bv0_sb = const.tile([64, HPC], f32, tag="bv0")
        nc.sync.dma_start(out=bv0_sb, in_=bv_d[:].rearrange("(h p) -> p h", p=64))
        bv_sb = const.tile([64, HPC], f32, tag="bv")
        nc.vector.tensor_copy(bv_sb, bv0_sb)  # pre-touch: downstream deps stay DVE-local

        onesrow_sb = const.tile([1, QT], bf16, tag="onesrow")
        nc.vector.memset(onesrow_sb, 1.0)
        ones_sb = const.tile([64, 1], f32, tag="ones")
        nc.vector.memset(ones_sb, 1.0)

        qT_sb = big.tile([128, 2, S], bf16, tag="qT")   # pair t: head 2t rows 0:64, 2t+1 rows 64:128
        kT_sb = big.tile([128, 2, S], bf16, tag="kT")
        # v per head: [128 keys, NKT chunks, 64+1] with ones column at 64
        v_sb = [big.tile([128, NKT, DH + 1], bf16, tag=f"v{h}", name=f"v{h}") for h in range(HPC)]
        z_sb = [big.tile([64, S], bf16, tag=f"z{h}", name=f"z{h}") for h in range(HPC)]
        nrm_sb = big.tile([128, 2, S], bf16, tag="nrm")

        # ---- Phase B: QKV projections ----
        with tc.tile_pool(name="pbqk", bufs=3, space="PSUM") as pbqk, \
             tc.tile_pool(name="pbv", bufs=3, space="PSUM") as pbv:
            for t in range(2):
                for st in range(NQT):
                    ps = pbqk.tile([128, QT], f32, tag="qk")
                    for c in range(NDC):
                        nc.tensor.matmul(ps, wq_sb[:, c, t * 128:(t + 1) * 128],
                                         xt_sb[:, c, st * QT:(st + 1) * QT],
                                         start=(c == 0), stop=False)
                    nc.tensor.matmul(ps, bqr_sb[:, t * 128:(t + 1) * 128],
                                     onesrow_sb, start=False, stop=True)
                    nc.vector.tensor_copy(out=qT_sb[:, t, st * QT:(st + 1) * QT], in_=ps)
            for t in range(2):
                for st in range(NQT):
                    ps = pbqk.tile([128, QT], f32, tag="qk")
                    for c in range(NDC):
                        nc.tensor.matmul(ps, wk_sb[:, c, t * 128:(t + 1) * 128],
                                         xt_sb[:, c, st * QT:(st + 1) * QT],
                                         start=(c == 0), stop=False)
                    nc.tensor.matmul(ps, bkr_sb[:, t * 128:(t + 1) * 128],
                                     onesrow_sb, start=False, stop=True)
                    nc.vector.tensor_copy(out=kT_sb[:, t, st * QT:(st + 1) * QT], in_=ps)
            for st in range(NKT):
                ps = pbv.tile([128, CW], f32, tag="v")
                for c in range(NDC):
                    nc.tensor.matmul(ps, xt_sb[:, c, st * 128:(st + 1) * 128],
                                     wv_sb[:, c, :],
                                     start=(c == 0), stop=(c == NDC - 1))
                for h in range(HPC):
                    nc.vector.tensor_copy(out=v_sb[h][:, st, 0:DH],
                                          in_=ps[:, h * DH:(h + 1) * DH])
            for h in range(HPC):
                nc.vector.memset(v_sb[h][:, :, DH:DH + 1], 1.0)

        # ---- Phase C: attention (k-major scores, exp, AV with ones row) ----
        with tc.tile_pool(name="psc", bufs=2, space="PSUM") as psc, \
             tc.tile_pool(name="pav", bufs=4, space="PSUM") as pav, \
             tc.tile_pool(name="pexp", bufs=4) as pexp, \
             tc.tile_pool(name="prs", bufs=4) as prs:
            for qt in range(NQT):
                av = [pav.tile([DH + 1, QT], f32, tag="av", name=f"av{qt}_{i}") for i in range(HPC)]
                for kt in range(NKT):
                    for t in range(2):
                        sps = psc.tile([128, 2 * QT], f32, tag="s")
                        for o in range(2):
                            nc.tensor.matmul(
                                sps[:, o * QT:(o + 1) * QT],
                                kT_sb[64 * o:64 * (o + 1), t, kt * 128:(kt + 1) * 128],
                                qT_sb[64 * o:64 * (o + 1), t, qt * QT:(qt + 1) * QT],
                                start=True, stop=True)
                        e_sb = pexp.tile([128, 2 * QT], bf16, tag="e")
                        nc.scalar.activation(e_sb, sps, AF.Exp)
                        for o in range(2):
                            h = 2 * t + o
                            nc.tensor.matmul(av[h], v_sb[h][:, kt, :],
                                             e_sb[:, o * QT:(o + 1) * QT],
                                             start=(kt == 0), stop=(kt == NKT - 1))
                for h in range(HPC):
                    nc.vector.tensor_copy(out=z_sb[h][:, qt * QT:(qt + 1) * QT],
                                          in_=av[h][0:DH, :])
                    rs = prs.tile([DH + 1, QT], f32, tag="rs")
                    nc.vector.tensor_copy(out=rs[DH:DH + 1, :], in_=av[h][DH:DH + 1, :])
                    nc.sync.dma_start(out=rs_d[h * NQT + qt:h * NQT + qt + 1, :],
                                      in_=rs[DH:DH + 1, :])

        # ---- Phase D: softmax normalize + GroupNorm ----
        with tc.tile_pool(name="pd", bufs=1) as pd, \
             tc.tile_pool(name="pst", bufs=4, space="PSUM") as pst:
            rs_flat = pd.tile([HPC, S], f32, tag="rsf")
            nc.sync.dma_start(out=rs_flat,
                              in_=rs_d[:, :].rearrange("(h qt) q -> h (qt q)", h=HPC))
            rs_rec = pd.tile([HPC, S], f32, tag="rsr")
            nc.vector.reciprocal_approx_fast(rs_rec, rs_flat)
            nc.sync.dma_start(out=rc_d[:, :], in_=rs_rec)

            mv = []
            for h in range(HPC):
                rb = pd.tile([64, S], f32, tag="rb", bufs=2, name=f"rb{h}")
                nc.gpsimd.dma_start(out=rb, in_=rc_d[h:h + 1, :].to_broadcast([64, S]))
                # z <- z * recip  (softmax normalization), in place
                nc.vector.tensor_mul(z_sb[h], z_sb[h], rb)
                st_t = pd.tile([64, NQT, 6], f32, tag=f"bn{h}")
                for st in range(NQT):
                    nc.vector.bn_stats(out=st_t[:, st, :],
                                       in_=z_sb[h][:, st * QT:(st + 1) * QT])
                mvh = pd.tile([64, 2], f32, tag=f"mv{h}")
                nc.vector.bn_aggr(out=mvh, in_=st_t)
                mv.append(mvh)

            scg = pd.tile([1, HPC, 3], f32, tag="scg")
            for h in range(HPC):
                stk = pd.tile([64, 3], f32, tag=f"stk{h}")
                nc.vector.tensor_add(stk[:, 0:1], mv[h][:, 0:1], bv_sb[:, h:h + 1])
                nc.vector.tensor_copy(stk[:, 1:2], mv[h][:, 1:2])
                nc.vector.tensor_mul(stk[:, 2:3], stk[:, 0:1], stk[:, 0:1])
                stp = pst.tile([1, 3], f32, tag="stp")
                nc.tensor.matmul(stp, ones_sb, stk, start=True, stop=True)
                nc.vector.tensor_copy(scg[:, h, :], stp)

            e3 = pd.tile([1, HPC, 3], f32, tag="e3")
            nc.vector.tensor_scalar(out=e3, in0=scg, scalar1=1.0 / 64.0,
                                    scalar2=None, op0=ALU.mult)
            m2 = pd.tile([1, HPC], f32, tag="m2")
            nc.vector.tensor_mul(m2, e3[:, :, 0], e3[:, :, 0])
            vr = pd.tile([1, HPC], f32, tag="vr")
            nc.vector.tensor_add(vr, e3[:, :, 1], e3[:, :, 2])
            nc.vector.tensor_tensor(out=vr, in0=vr, in1=m2, op=ALU.subtract)
            sd = pd.tile([1, HPC], f32, tag="sd")
            eps_t = pd.tile([1, 1], f32, tag="eps")
            nc.vector.memset(eps_t, EPS)
            nc.scalar.activation(sd, vr, AF.Sqrt, bias=eps_t)
            rstd = pd.tile([1, HPC], f32, tag="rstd")
            nc.vector.reciprocal(rstd, sd)
            nc.sync.dma_start(out=sc_d[:].rearrange("(a b) -> a b", a=2)[0:1, :],
                              in_=e3[:, :, 0])
            nc.sync.dma_start(out=sc_d[:].rearrange("(a b) -> a b", a=2)[1:2, :],
                              in_=rstd)

            mr = pd.tile([64, 8], f32, tag="mr")
            nc.gpsimd.dma_start(
                out=mr, in_=sc_d[:].rearrange("(a b) -> a b", a=1).to_broadcast([64, 8]))
            s1 = pd.tile([64, HPC], f32, tag="s1")
            nc.vector.tensor_tensor(out=s1, in0=mr[:, 0:HPC], in1=bv_sb, op=ALU.subtract)

            for h in range(HPC):
                t, o = h // 2, h % 2
                dst = nrm_sb[0:64, t, :] if o == 0 else z_sb[h]
                nc.vector.tensor_scalar(out=dst, in0=z_sb[h],
                                        scalar1=s1[:, h:h + 1],
                                        scalar2=mr[:, 4 + h:5 + h],
                                        op0=ALU.subtract, op1=ALU.mult)
            for t in range(2):
                nc.sync.dma_start(out=nrm_sb[64:128, t, :], in_=z_sb[2 * t + 1])

        # ---- Phase E: out-projection (yT layout), streamed to cc_in ----
        with tc.tile_pool(name="pe", bufs=4, space="PSUM") as pe, \
             tc.tile_pool(name="pystage", bufs=2) as pystage:
            for nt in range(NDC):
                ystage = pystage.tile([128, S], f32, tag="ys", name=f"ys{nt}")
                for st in range(NQT):
                    yp = pe.tile([128, QT], f32, tag="y")
                    for t in range(2):
                        nc.tensor.matmul(yp, wo_sb[:, t, nt * 128:(nt + 1) * 128],
                                         nrm_sb[:, t, st * QT:(st + 1) * QT],
                                         start=(t == 0), stop=False)
                    nc.tensor.matmul(yp, bor_sb[:, nt * 128:(nt + 1) * 128],
                                     onesrow_sb, start=False, stop=True)
                    nc.vector.tensor_copy(out=ystage[:, st * QT:(st + 1) * QT], in_=yp)
                nc.sync.dma_start(out=cc_in[nt, :, :], in_=ystage)

        # ---- Phase F: AllReduce over the 4 cores of this batch ----
        if with_collective:
            nc.gpsimd.collective_compute(
                "AllReduce", ALU.add,
                replica_groups=[[0, 1, 2, 3], [4, 5, 6, 7]],
                ins=[cc_in[:].opt()],
                outs=[cc_out[:].opt()],
            )
            nc.sync.dma_start(out=y_d[:, :, :], in_=cc_out[:, :, :])
        else:
            nc.sync.dma_start(out=y_d[:, :, :], in_=cc_in[:, :, :])

    nc.compile()
    return nc


def _get_nc():
    if "nc" not in _cache:
        _cache["nc"] = _build()
    return _cache["nc"]


def kernel(x, Wq, bq, Wk, bk, Wv, bv, Wo, bo, lq1, lk1, lq2, lk2, gn_w, gn_b):
    from concourse.bass_utils import run_bass_kernel_spmd

    x = np.asarray(x, np.float32)
    lam = (np.exp((np.asarray(lq1) * np.asarray(lk1)).sum(-1))
           - np.exp((np.asarray(lq2) * np.asarray(lk2)).sum(-1)) + LAMBDA_INIT)  # [H]
    qscale = (DH ** -0.5) * lam
    Wq_eff = (np.asarray(Wq).reshape(D, H, DH) * qscale[None, :, None]).reshape(D, D)
    bq_eff = (np.asarray(bq).reshape(H, DH) * qscale[:, None]).reshape(D)
    gw = np.asarray(gn_w).reshape(D)
    gb = np.asarray(gn_b).reshape(D)
    Wo_eff = np.asarray(Wo) * gw[:, None]
    bo_eff = np.asarray(bo) + gb @ np.asarray(Wo)

    xT = np.ascontiguousarray(x.transpose(0, 2, 1))  # [B, D, S]
    bf = ml_dtypes.bfloat16
    zeros_bo = np.zeros(D, np.float32)

    in_maps = []
    for c in range(N_CORES):
        b, hg = c // 4, c % 4
        cs = slice(CW * hg, CW * (hg + 1))
        in_maps.append({
            "xt": np.ascontiguousarray(xT[b]).astype(bf),
            "wq": np.ascontiguousarray(Wq_eff[:, cs]).astype(bf),
            "wk": np.ascontiguousarray(np.asarray(Wk)[:, cs]).astype(bf),
            "wv": np.ascontiguousarray(np.asarray(Wv)[:, cs]).astype(bf),
            "wo": np.ascontiguousarray(Wo_eff[cs, :]).astype(bf),
            "bq": np.ascontiguousarray(bq_eff[cs]).astype(bf),
            "bk": np.ascontiguousarray(np.asarray(bk)[cs]).astype(bf),
            "bv": np.ascontiguousarray(np.asarray(bv)[cs]).astype(np.float32),
            "bo": (bo_eff if hg == 0 else zeros_bo).astype(bf),
        })

    nc = _get_nc()
    res = run_bass_kernel_spmd(nc, in_maps, core_ids=list(range(N_CORES)))
    outs = res.results
    y0 = np.asarray(outs[0]["y"]).reshape(D, S)
    y1 = np.asarray(outs[4]["y"]).reshape(D, S)
    y = np.stack([y0.T, y1.T])  # [B, S, D]
    return np.ascontiguousarray(y).astype(np.float32)


# revision 11
# speedup vs baseline: 1.6781x; 1.6781x over previous
"""Multi-head differential attention on 8 Trainium2 NeuronCores.

Sharding: core c -> batch c//4, head-group c%4 (4 of 16 heads).
Per core: QKV projection for its heads, k-major attention (scores
transposed so softmax sums come from a ones-row in V via the AV matmul),
per-(batch,head) GroupNorm, then an AllGather of the normalized heads
(bf16, 1 MB) and a column-parallel out-projection: each core produces a
256-column slice of the output, assembled on host.

Host side folds: lambda and softmax scale into Wq/bq; GroupNorm affine
into Wo/bo.  x is pre-transposed per batch and cast to bf16 so all
matmuls run at 1 cycle/row.
"""

import numpy as np
import ml_dtypes

B, S, D, H, DH = 2, 2048, 1024, 16, 64
HPC = 4            # heads per core
CW = HPC * DH      # attention columns per core (256)
EPS = 1e-5
LAMBDA_INIT = 0.8
N_CORES = 8

_cache = {}


def _build(with_collective=True):
    from contextlib import ExitStack
    import concourse.bass as bass
    from concourse import bacc
    import concourse.tile as tile
    import concourse.mybir as mybir

    f32 = mybir.dt.float32
    bf16 = mybir.dt.bfloat16
    AF = mybir.ActivationFunctionType
    ALU = mybir.AluOpType

    nc = bacc.Bacc("TRN2", target_bir_lowering=False, debug=False,
                   num_devices=N_CORES)

    xt_d = nc.dram_tensor("xt", [D, S], bf16, kind="ExternalInput")
    wq_d = nc.dram_tensor("wq", [D, CW], bf16, kind="ExternalInput")
    wk_d = nc.dram_tensor("wk", [D, CW], bf16, kind="ExternalInput")
    wv_d = nc.dram_tensor("wv", [D, CW], bf16, kind="ExternalInput")
    # wo: gathered-row layout [(g t o p), quarter-cols]
    wo_d = nc.dram_tensor("wo", [D, CW], bf16, kind="ExternalInput")
    bq_d = nc.dram_tensor("bq", [CW], bf16, kind="ExternalInput")
    bk_d = nc.dram_tensor("bk", [CW], bf16, kind="ExternalInput")
    bv_d = nc.dram_tensor("bv", [CW], f32, kind="ExternalInput")
    bo_d = nc.dram_tensor("bo", [CW], bf16, kind="ExternalInput")
    y_d = nc.dram_tensor("y", [2, 128, S], f32, kind="ExternalOutput")

    ag_in = nc.dram_tensor("ag_in", [128, 2, S], bf16)
    ag_out = nc.dram_tensor("ag_out", [4, 128, 2, S], bf16)
    rs_d = nc.dram_tensor("rs_scratch", [HPC, S], f32)
    rc_d = nc.dram_tensor("rc_scratch", [HPC, S], f32)
    sc_d = nc.dram_tensor("sc_scratch", [2, 4], f32)   # per pair: M0 M1 r0 r1

    NQT = 4          # query tiles of 512
    QT = 512
    NKT = 16         # key tiles of 128
    NDC = 8          # d-chunks of 128

    with ExitStack() as ctx:
        tc = ctx.enter_context(tile.TileContext(nc))
        const = ctx.enter_context(tc.tile_pool(name="const", bufs=1))
        big = ctx.enter_context(tc.tile_pool(name="big", bufs=1))

        # ---- constants ----
        wq_sb = const.tile([128, NDC, CW], bf16, tag="wq")
        wk_sb = const.tile([128, NDC, CW], bf16, tag="wk")
        wv_sb = const.tile([128, NDC, CW], bf16, tag="wv")
        nc.sync.dma_start(out=wq_sb, in_=wq_d[:, :].rearrange("(c p) n -> p c n", p=128))
        nc.sync.dma_start(out=wk_sb, in_=wk_d[:, :].rearrange("(c p) n -> p c n", p=128))
        nc.sync.dma_start(out=wv_sb, in_=wv_d[:, :].rearrange("(c p) n -> p c n", p=128))
        wo_sb = const.tile([128, NDC, CW], bf16, tag="wo")
        nc.sync.dma_start(out=wo_sb, in_=wo_d[:, :].rearrange("(c p) n -> p c n", p=128))

        bqr_sb = const.tile([1, CW], bf16, tag="bqr")
        bkr_sb = const.tile([1, CW], bf16, tag="bkr")
        bor_sb = const.tile([1, CW], bf16, tag="bor")
        nc.sync.dma_start(out=bqr_sb, in_=bq_d[:].rearrange("(a n) -> a n", a=1))
        nc.sync.dma_start(out=bkr_sb, in_=bk_d[:].rearrange("(a n) -> a n", a=1))
        nc.sync.dma_start(out=bor_sb, in_=bo_d[:].rearrange("(a n) -> a n", a=1))
        bv0_sb = const.tile([64, HPC], f32, tag="bv0")
        nc.sync.dma_start(out=bv0_sb, in_=bv_d[:].rearrange("(h p) -> p h", p=64))
        bv_sb = const.tile([64, HPC], f32, tag="bv")
        nc.vector.tensor_copy(bv_sb, bv0_sb)  # pre-touch: keep deps DVE-local

        onesrow_sb = const.tile([1, QT], bf16, tag="onesrow")
        nc.vector.memset(onesrow_sb, 1.0)
        ones_sb = const.tile([64, 1], f32, tag="ones")
        nc.vector.memset(ones_sb, 1.0)

        qT_sb = big.tile([128, 2, S], bf16, tag="qT")   # pair t: head 2t rows 0:64
        kT_sb = big.tile([128, 2, S], bf16, tag="kT")
        v_sb = [big.tile([128, NKT, DH + 1], bf16, tag=f"v{h}", name=f"v{h}")
                for h in range(HPC)]
        z_sb = [big.tile([DH + 1, S], f32, tag=f"z{h}", name=f"z{h}")
                for h in range(HPC)]
        nrm_sb = big.tile([128, 2, S], bf16, tag="nrm")

        # ---- Phase B: QKV projections (pair 0 first so attention starts early)
        with tc.tile_pool(name="pxt", bufs=1) as pxt, \
             tc.tile_pool(name="pbqk", bufs=3, space="PSUM") as pbqk, \
             tc.tile_pool(name="pbv", bufs=3, space="PSUM") as pbv:
            xt_sb = [pxt.tile([128, S], bf16, tag=f"xt{c}", name=f"xt{c}")
                     for c in range(NDC)]
            for c in range(NDC):
                nc.sync.dma_start(out=xt_sb[c], in_=xt_d[c * 128:(c + 1) * 128, :])

            def qk_proj(t, w_sb, br_sb, dst):
                for st in range(NQT):
                    ps = pbqk.tile([128, QT], f32, tag="qk", name=f"qk{t}{st}")
                    for c in range(NDC):
                        nc.tensor.matmul(ps, w_sb[:, c, t * 128:(t + 1) * 128],
                                         xt_sb[c][:, st * QT:(st + 1) * QT],
                                         start=(c == 0), stop=False)
                    nc.tensor.matmul(ps, br_sb[:, t * 128:(t + 1) * 128],
                                     onesrow_sb, start=False, stop=True)
                    nc.vector.tensor_copy(out=dst[:, t, st * QT:(st + 1) * QT], in_=ps)

            qk_proj(0, wq_sb, bqr_sb, qT_sb)
            qk_proj(0, wk_sb, bkr_sb, kT_sb)
            for st in range(NKT):
                ps = pbv.tile([128, CW], f32, tag="v", name=f"vv{st}")
                for c in range(NDC):
                    nc.tensor.matmul(ps, xt_sb[c][:, st * 128:(st + 1) * 128],
                                     wv_sb[:, c, :],
                                     start=(c == 0), stop=(c == NDC - 1))
                for h in range(HPC):
                    nc.vector.tensor_copy(out=v_sb[h][:, st, 0:DH],
                                          in_=ps[:, h * DH:(h + 1) * DH])
            for h in range(HPC):
                nc.vector.memset(v_sb[h][:, :, DH:DH + 1], 1.0)
            qk_proj(1, wq_sb, bqr_sb, qT_sb)
            qk_proj(1, wk_sb, bkr_sb, kT_sb)

        # ---- Phase C+D: attention per head-pair, GN overlapped ----
        with tc.tile_pool(name="psc", bufs=2, space="PSUM") as psc, \
             tc.tile_pool(name="pav", bufs=3, space="PSUM") as pav, \
             tc.tile_pool(name="pexp", bufs=3) as pexp, \
             tc.tile_pool(name="pd", bufs=1) as pd:
            for t in range(2):
                h0, h1 = 2 * t, 2 * t + 1
                for qt in range(NQT):
                    av0 = pav.tile([DH + 1, QT], f32, tag="av", name=f"av{t}{qt}a")
                    av1 = pav.tile([DH + 1, QT], f32, tag="av", name=f"av{t}{qt}b")
                    for kt in range(NKT):
                        sps = psc.tile([128, 2 * QT], f32, tag="s", name=f"s{t}{qt}{kt}")
                        for o in range(2):
                            nc.tensor.matmul(
                                sps[:, o * QT:(o + 1) * QT],
                                kT_sb[64 * o:64 * (o + 1), t, kt * 128:(kt + 1) * 128],
                                qT_sb[64 * o:64 * (o + 1), t, qt * QT:(qt + 1) * QT],
                                start=True, stop=True)
                        e_sb = pexp.tile([128, 2 * QT], bf16, tag="e", name=f"e{t}{qt}{kt}")
                        nc.scalar.activation(e_sb, sps, AF.Exp)
                        nc.tensor.matmul(av0, v_sb[h0][:, kt, :], e_sb[:, 0:QT],
                                         start=(kt == 0), stop=(kt == NKT - 1))
                        nc.tensor.matmul(av1, v_sb[h1][:, kt, :], e_sb[:, QT:2 * QT],
                                         start=(kt == 0), stop=(kt == NKT - 1))
                    nc.vector.tensor_copy(out=z_sb[h0][:, qt * QT:(qt + 1) * QT], in_=av0)
                    nc.vector.tensor_copy(out=z_sb[h1][:, qt * QT:(qt + 1) * QT], in_=av1)

                # ---- GroupNorm for this pair (overlaps next pair's attention)
                for h in (h0, h1):
                    nc.sync.dma_start(out=rs_d[h:h + 1, :], in_=z_sb[h][DH:DH + 1, :])
                rs_flat = pd.tile([2, S], f32, tag="rsf", bufs=2, name=f"rsf{t}")
                nc.sync.dma_start(out=rs_flat, in_=rs_d[2 * t:2 * t + 2, :])
                rs_rec = pd.tile([2, S], f32, tag="rsr", bufs=2, name=f"rsr{t}")
                nc.vector.reciprocal_approx_fast(rs_rec, rs_flat)
                nc.sync.dma_start(out=rc_d[2 * t:2 * t + 2, :], in_=rs_rec)

                mv = {}
                for h in (h0, h1):
                    rb = pd.tile([64, S], f32, tag="rb", bufs=2, name=f"rb{h}")
                    nc.gpsimd.dma_start(out=rb,
                                        in_=rc_d[h:h + 1, :].to_broadcast([64, S]))
                    nc.vector.tensor_mul(z_sb[h][0:DH, :], z_sb[h][0:DH, :], rb)
                    st_t = pd.tile([64, NQT, 6], f32, tag="bn", bufs=2, name=f"bn{h}")
                    for st in range(NQT):
                        nc.vector.bn_stats(out=st_t[:, st, :],
                                           in_=z_sb[h][0:DH, st * QT:(st + 1) * QT])
                    mvh = pd.tile([64, 2], f32, tag="mv", bufs=2, name=f"mv{h}")
                    nc.vector.bn_aggr(out=mvh, in_=st_t)
                    mv[h] = mvh

                scg = pd.tile([1, 2, 3], f32, tag="scg", bufs=2, name=f"scg{t}")
                for i, h in enumerate((h0, h1)):
                    stk = pd.tile([64, 3], f32, tag="stk", bufs=2, name=f"stk{h}")
                    nc.vector.tensor_add(stk[:, 0:1], mv[h][:, 0:1], bv_sb[:, h:h + 1])
                    nc.vector.tensor_copy(stk[:, 1:2], mv[h][:, 1:2])
                    nc.vector.tensor_mul(stk[:, 2:3], stk[:, 0:1], stk[:, 0:1])
                    stp = pav.tile([1, 3], f32, tag="stp", bufs=1, name=f"stp{h}")
                    nc.tensor.matmul(stp, ones_sb, stk, start=True, stop=True)
                    nc.vector.tensor_copy(scg[:, i, :], stp)

                e3 = pd.tile([1, 2, 3], f32, tag="e3", bufs=2, name=f"e3{t}")
                nc.vector.tensor_scalar(out=e3, in0=scg, scalar1=1.0 / 64.0,
                                        scalar2=None, op0=ALU.mult)
                m2 = pd.tile([1, 2], f32, tag="m2", bufs=2, name=f"m2{t}")
                nc.vector.tensor_mul(m2, e3[:, :, 0], e3[:, :, 0])
                vr = pd.tile([1, 2], f32, tag="vr", bufs=2, name=f"vr{t}")
                nc.vector.tensor_add(vr, e3[:, :, 1], e3[:, :, 2])
                nc.vector.tensor_tensor(out=vr, in0=vr, in1=m2, op=ALU.subtract)
                sd = pd.tile([1, 2], f32, tag="sd", bufs=2, name=f"sd{t}")
                eps_t = pd.tile([1, 1], f32, tag="eps", bufs=2, name=f"eps{t}")
                nc.vector.memset(eps_t, EPS)
                nc.scalar.activation(sd, vr, AF.Sqrt, bias=eps_t)
                rstd = pd.tile([1, 2], f32, tag="rstd", bufs=2, name=f"rstd{t}")
                nc.vector.reciprocal(rstd, sd)
                nc.sync.dma_start(out=sc_d[t:t + 1, 0:2], in_=e3[:, :, 0])
                nc.sync.dma_start(out=sc_d[t:t + 1, 2:4], in_=rstd)

                mr = pd.tile([64, 4], f32, tag="mr", bufs=2, name=f"mr{t}")
                nc.gpsimd.dma_start(out=mr,
                                    in_=sc_d[t:t + 1, :].to_broadcast([64, 4]))
                s1 = pd.tile([64, 2], f32, tag="s1", bufs=2, name=f"s1{t}")
                nc.vector.tensor_tensor(out=s1, in0=mr[:, 0:2],
                                        in1=bv_sb[:, h0:h0 + 2], op=ALU.subtract)

                for i, h in enumerate((h0, h1)):
                    dst = nrm_sb[0:64, t, :] if i == 0 else z_sb[h][0:DH, :]
                    nc.vector.tensor_scalar(out=dst, in0=z_sb[h][0:DH, :],
                                            scalar1=s1[:, i:i + 1],
                                            scalar2=mr[:, 2 + i:3 + i],
                                            op0=ALU.subtract, op1=ALU.mult)
                # odd head: cross-partition move (and f32->bf16 cast) via DMA
                nc.gpsimd.dma_start(out=nrm_sb[64:128, t, :], in_=z_sb[h1][0:DH, :])

        # ---- Phase E: AllGather heads, column-parallel out-projection ----
        nc.sync.dma_start(out=ag_in[:, :, :], in_=nrm_sb)
        if with_collective:
            nc.gpsimd.collective_compute(
                "AllGather", ALU.bypass,
                replica_groups=[[0, 1, 2, 3], [4, 5, 6, 7]],
                ins=[ag_in[:].opt()],
                outs=[ag_out[:].opt()],
            )
        else:
            for g in range(4):
                nc.sync.dma_start(out=ag_out[g], in_=ag_in[:, :, :])

        with tc.tile_pool(name="pg", bufs=1) as pg, \
             tc.tile_pool(name="pe", bufs=4, space="PSUM") as pe, \
             tc.tile_pool(name="pystage", bufs=2) as pystage:
            nrmg_sb = pg.tile([128, NDC, S], bf16, tag="nrmg")
            for g in range(4):
                nc.sync.dma_start(out=nrmg_sb[:, 2 * g:2 * g + 2, :],
                                  in_=ag_out[g, :, :, :])
            for nt in range(2):
                ystage = pystage.tile([128, S], f32, tag="ys", name=f"ys{nt}")
                for st in range(NQT):
                    yp = pe.tile([128, QT], f32, tag="y", name=f"yp{nt}{st}")
                    for c in range(NDC):
                        nc.tensor.matmul(yp, wo_sb[:, c, nt * 128:(nt + 1) * 128],
                                         nrmg_sb[:, c, st * QT:(st + 1) * QT],
                                         start=(c == 0), stop=False)
                    nc.tensor.matmul(yp, bor_sb[:, nt * 128:(nt + 1) * 128],
                                     onesrow_sb, start=False, stop=True)
                    nc.scalar.activation(ystage[:, st * QT:(st + 1) * QT], yp, AF.Copy)
                nc.sync.dma_start(out=y_d[nt, :, :], in_=ystage)

    nc.compile()
    return nc


def _get_nc():
    if "nc" not in _cache:
        _cache["nc"] = _build()
    return _cache["nc"]


def _host_prep(x, Wq, bq, Wk, bk, Wv, bv, Wo, bo, lq1, lk1, lq2, lk2, gn_w, gn_b):
    x = np.asarray(x, np.float32)
    lam = (np.exp((np.asarray(lq1) * np.asarray(lk1)).sum(-1))
           - np.exp((np.asarray(lq2) * np.asarray(lk2)).sum(-1)) + LAMBDA_INIT)
    qscale = (DH ** -0.5) * lam
    Wq_eff = (np.asarray(Wq).reshape(D, H, DH) * qscale[None, :, None]).reshape(D, D)
    bq_eff = (np.asarray(bq).reshape(H, DH) * qscale[:, None]).reshape(D)
    gw = np.asarray(gn_w).reshape(D)
    gb = np.asarray(gn_b).reshape(D)
    Wo_eff = np.asarray(Wo) * gw[:, None]
    bo_eff = np.asarray(bo) + gb @ np.asarray(Wo)

    # Out-proj rows in "gathered" order: AllGather output is the 4 cores'
    # [128,2,S] pair-stacked tiles concatenated, i.e. chunk (g,t) holds
    # original row (4g + 2t + o)*64 + dh at partition o*64+dh.  Wo_eff rows
    # are already in (g, t, o, dh) order when viewed as [4,2,2,64,D].
    Wo_g = np.ascontiguousarray(Wo_eff.reshape(4, 2, 2, DH, D).reshape(D, D))

    xT = np.ascontiguousarray(x.transpose(0, 2, 1))  # [B, D, S]
    bf = ml_dtypes.bfloat16

    in_maps = []
    for c in range(N_CORES):
        b, hg = c // 4, c % 4
        cs = slice(CW * hg, CW * (hg + 1))
        in_maps.append({
            "xt": np.ascontiguousarray(xT[b]).astype(bf),
            "wq": np.ascontiguousarray(Wq_eff[:, cs]).astype(bf),
            "wk": np.ascontiguousarray(np.asarray(Wk)[:, cs]).astype(bf),
            "wv": np.ascontiguousarray(np.asarray(Wv)[:, cs]).astype(bf),
            "wo": np.ascontiguousarray(Wo_g[:, cs]).astype(bf),
            "bq": np.ascontiguousarray(bq_eff[cs]).astype(bf),
            "bk": np.ascontiguousarray(np.asarray(bk)[cs]).astype(bf),
            "bv": np.ascontiguousarray(np.asarray(bv)[cs]).astype(np.float32),
            "bo": np.ascontiguousarray(bo_eff[cs]).astype(bf),
        })
    return in_maps


def _host_gather(outs):
    # core c=4b+hg produced output columns [256*hg, 256*(hg+1)) as [2,128,S]
    yT = np.empty((B, D, S), np.float32)
    for b in range(B):
        for hg in range(4):
            q = np.asarray(outs[4 * b + hg]["y"]).reshape(CW, S)
            yT[b, CW * hg:CW * (hg + 1), :] = q
    return np.ascontiguousarray(yT.transpose(0, 2, 1))


def kernel(x, Wq, bq, Wk, bk, Wv, bv, Wo, bo, lq1, lk1, lq2, lk2, gn_w, gn_b):
    from concourse.bass_utils import run_bass_kernel_spmd

    in_maps = _host_prep(x, Wq, bq, Wk, bk, Wv, bv, Wo, bo,
                         lq1, lk1, lq2, lk2, gn_w, gn_b)
    nc = _get_nc()
    res = run_bass_kernel_spmd(nc, in_maps, core_ids=list(range(N_CORES)))
    return _host_gather(res.results)
